# revision 27
# baseline (speedup 1.0000x reference)
"""Trainium2 Bass kernel: patch-conv (Conv2d C3->E768, k4 s4) + giant linear.

y[b, eo] = sum_K flat[b, K] * wlin[eo, K] + blin[eo],
flat[b, e*256+p] = conv[b, e, p] (+ bconv[e]), K = 196608.

Reassociated algorithm (matrix-chain reordering, all FLOPs on device):
    y[b,eo] = sum_{p,cij} xp[cij,p,b] * W2[p,cij,eo]
    W2[p,cij,eo] = sum_e wca[cij,e] * wlin[eo, e*256+p]
where xp is the im2col of x (pure index remap, row 48 = ones) and wca is
wconv reshaped [49, E] (row 48 = bconv). This computes the identical
function with 19.7 GFLOP instead of 82 GFLOP, and reads wlin exactly once.

Sharding (8 cores): shard the patch index p (32 patches/core). Each core:
  - streams its wlin slice (host re-layout wlp[e_r, (p, ech, eo)] bf16,
    37.75 MB -- the HBM roofline term) on the sync HWDGE queue,
  - W2-mm per patch pair: the even patch uses stationary wce [128,49]
    (PSUM rows 0-48); the odd patch uses wce_odd [128,113] whose first 64
    columns are zero, so its result lands at PSUM rows 64-112 of the SAME
    bank while rows 0-63 accumulate +0 (never-written-region trick),
  - copies the pair's W2 [113, 768] to SBUF bf16 (rows 49-63 are zeros),
  - final-mm contracts over all 113 rows at once: xpp2 packs the even
    patch's im2col at rows 0-48, zeros at 49-63, odd at 64-112 -> one
    matmul accumulates BOTH patches into y-partial PSUM (halves the
    final-mm column count vs per-patch matmuls).
  - The last two patches run as singles to minimize the end-of-stream
    serial chain. Output partial-y is cast bf16; host sums cores + blin.
All PSUM tiles are bank-sized (2048B): 4 y banks + 2x2 W2 banks = 8.
"""

import numpy as np
import ml_dtypes

B, C, H, W = 256, 3, 64, 64
P, Hp, Wp, NP = 4, 16, 16, 256
E = 768
CIJ = 49                  # 48 conv taps + 1 bias row
ROWS = 113                # packed pair rows: 49 even + 15 zero + 49 odd
NCORES = 8
PL = NP // NCORES         # 32 patches per core
NPAIR = PL // 2           # 16 patch pairs
NECH = E // 128           # 6 e-chunks in the W2 contraction
PCOL = NECH * E           # 4608 wlp columns per patch

_CACHE = {}


def _build_bass():
    import concourse.bass as bass
    import concourse.mybir as mybir
    import concourse.tile as tile
    from contextlib import ExitStack

    dt = mybir.dt
    nc = bass.Bass()
    wce_d = nc.dram_tensor("wce", [128, NECH * CIJ], dt.bfloat16, kind="ExternalInput")
    wco_d = nc.dram_tensor("wco", [128, NECH * ROWS], dt.bfloat16, kind="ExternalInput")
    xpp_d = nc.dram_tensor("xpp", [ROWS, NPAIR * B], dt.bfloat16, kind="ExternalInput")
    wlp_d = nc.dram_tensor("wlp", [128, PL * PCOL], dt.bfloat16, kind="ExternalInput")
    out_d = nc.dram_tensor("y", [B, E], dt.bfloat16, kind="ExternalOutput")

    with tile.TileContext(nc) as tc, ExitStack() as ctx:
        singles = ctx.enter_context(tc.tile_pool(name="singles", bufs=1))
        # Everything rides the two HWDGE rings (sync for the wlp stream,
        # scalar for the small loads and outputs); zero SWDGE traffic.
        wce = singles.tile([128, NECH * CIJ], dt.bfloat16)
        nc.scalar.dma_start(out=wce[:], in_=wce_d[:])
        wco = singles.tile([128, NECH * ROWS], dt.bfloat16)
        nc.scalar.dma_start(out=wco[:], in_=wco_d[:])
        xpp = singles.tile([ROWS, NPAIR * B], dt.bfloat16)
        half = NPAIR * B // 2
        nc.scalar.dma_start(out=xpp[:, 0:half], in_=xpp_d[:, 0:half])
        nc.scalar.dma_start(out=xpp[:, half:], in_=xpp_d[:, half:])

        # W2 staging ring: one [113, 768] bf16 tile per patch pair.
        w2_pool = ctx.enter_context(tc.tile_pool(name="w2ring", bufs=6))
        wlp_pool = ctx.enter_context(tc.tile_pool(name="wlp", bufs=4))
        out_pool = ctx.enter_context(tc.tile_pool(name="out_sb", bufs=1))

        # Warmup: absorb the wce/wco DMA-ready waits on throwaway matmuls so
        # the first real W2 matmul carries only the wlp(0) wait.
        with tc.tile_pool(name="psum_w", bufs=1, space="PSUM") as pwarm:
            wm = pwarm.tile([CIJ, CIJ], dt.float32)
            nc.tensor.matmul(
                wm[:], wce[:, 0:CIJ], wce[:, 0:CIJ], start=True, stop=True,
                skip_group_check=True,
            )
            wm2 = pwarm.tile([ROWS, CIJ], dt.float32, tag="wm2", name="wm2")
            nc.tensor.matmul(
                wm2[:], wco[:, 0:ROWS], wce[:, 0:CIJ], start=True, stop=True,
                skip_group_check=True,
            )

        with (
            tc.tile_pool(name="psum_y", bufs=1, space="PSUM") as pyp,
            tc.tile_pool(name="psum_w2", bufs=1, space="PSUM") as ppw,
        ):
            # y-partial accumulators: [128b x 512eo] + [128b x 256eo] per
            # b-half; each tile is a full PSUM bank.
            py = [
                [
                    pyp.tile([128, 512], dt.float32, tag=f"py{bh}0", name=f"py{bh}0"),
                    pyp.tile([128, 512], dt.float32, tag=f"py{bh}1", name=f"py{bh}1"),
                ]
                for bh in range(2)
            ]

            w2tiles = {}
            # Four bank-sized W2 accumulators (two pairs in flight).
            pw = [
                [
                    ppw.tile([ROWS, 512], dt.float32, tag=f"pa{j}", name=f"pa{j}"),
                    ppw.tile([ROWS, 512], dt.float32, tag=f"pb{j}", name=f"pb{j}"),
                ]
                for j in range(2)
            ]

            def w2_mms(j, wl, base, odd, first):
                # One patch's 12 accumulating matmuls. odd=True places the
                # result at PSUM rows 64-112 via the zero-padded stationary;
                # its rows 0-63 add +0 to the even patch's region.
                for ech in range(NECH):
                    if odd:
                        lhsT = wco[:, ech * ROWS : (ech + 1) * ROWS]
                        pa, pb = pw[j][0][:, 0:512], pw[j][1][:, 0:256]
                    else:
                        lhsT = wce[:, ech * CIJ : (ech + 1) * CIJ]
                        pa, pb = pw[j][0][0:CIJ, 0:512], pw[j][1][0:CIJ, 0:256]
                    nc.tensor.matmul(
                        pa, lhsT, wl[:, base + ech * E : base + ech * E + 512],
                        start=(first and ech == 0),
                        stop=(ech == NECH - 1),
                        skip_group_check=True,
                    )
                    nc.tensor.matmul(
                        pb, lhsT, wl[:, base + ech * E + 512 : base + ech * E + 768],
                        start=(first and ech == 0),
                        stop=(ech == NECH - 1),
                        skip_group_check=True,
                    )

            def w2_copies(pt, j, rows):
                # PSUM f32 -> SBUF bf16 cast-copies for row range `rows`.
                w2t = w2tiles[pt]
                lo, hi = rows
                if pt % 2 == 0:
                    nc.vector.tensor_copy(w2t[lo:hi, 0:512], pw[j][0][lo:hi, 0:512])
                    nc.vector.tensor_copy(w2t[lo:hi, 512:768], pw[j][1][lo:hi, 0:256])
                else:
                    nc.scalar.copy(w2t[lo:hi, 0:512], pw[j][0][lo:hi, 0:512])
                    nc.scalar.copy(w2t[lo:hi, 512:768], pw[j][1][lo:hi, 0:256])

            def w2_pair_block(pt, wl):
                w2tiles[pt] = w2_pool.tile([ROWS, E], dt.bfloat16, name="w2t")
                j = pt % 2
                # The odd matmul goes FIRST with start=True: PSUM pending-zero
                # marking covers only the writing instruction's partitions, so
                # the [113, *] odd output initializes the full row range
                # (rows 0-63 get its zero-stationary zeros); the even matmuls
                # then accumulate onto rows 0-48.
                w2_mms(j, wl, PCOL, odd=True, first=True)
                w2_mms(j, wl, 0, odd=False, first=False)
                # Copies obey the engine partition-window rule (base 0:
                # any size; base 32: <=32 rows; base 64: <=64). The middle
                # window re-writes rows 32-48 with identical values and
                # carries the PSUM zeros into rows 49-63, so the whole
                # [0:113] range of w2t is defined for the packed final.
                w2_copies(pt, j, (0, CIJ))
                w2_copies(pt, j, (32, 64))
                w2_copies(pt, j, (64, ROWS))

            def final_half(pt, rows, start, stop, pop):
                # Finals for a row range of pair pt's packed W2 tile.
                w2t = w2tiles.pop(pt) if pop else w2tiles[pt]
                lo, hi = rows
                for bh in range(2):
                    lhsT = xpp[lo:hi, pt * B + bh * 128 : pt * B + bh * 128 + 128]
                    nc.tensor.matmul(
                        py[bh][0][:, 0:512],
                        lhsT,
                        w2t[lo:hi, 0:512],
                        start=start, stop=stop, skip_group_check=True,
                    )
                    nc.tensor.matmul(
                        py[bh][1][:, 0:256],
                        lhsT,
                        w2t[lo:hi, 512:768],
                        start=start, stop=stop, skip_group_check=True,
                    )

            def final_block(pt):
                # One matmul pair per b-half contracts BOTH patches of the
                # pair (rows 0-48 even, 64-112 odd, 49-63 zeros).
                final_half(pt, (0, ROWS), start=(pt == 0), stop=False, pop=True)

            # wlp streams as one sequential 2-patch DMA chain on the sync
            # HWDGE queue (18432B per-partition lines, one packet each); the
            # final blocks run one pair behind W2 so the PE never stalls on
            # the PSUM->SBUF copy round-trip. The last pair is split into
            # 1-patch DMAs/matmul chains to shorten the end-of-stream chain.
            for pt in range(NPAIR - 1):
                wl = wlp_pool.tile([128, 2 * PCOL], dt.bfloat16, name="wl")
                nc.sync.dma_start(
                    out=wl[:], in_=wlp_d[:, pt * 2 * PCOL : (pt + 1) * 2 * PCOL]
                )
                w2_pair_block(pt, wl)
                if pt >= 1:
                    final_block(pt - 1)

            # Tail: last pair as two 1-patch DMAs and split finals, so the
            # serial chain after the last wlp packet is just the odd half's
            # 12 matmuls + copy + 4 finals.
            lt = NPAIR - 1
            w2tiles[lt] = w2_pool.tile([ROWS, E], dt.bfloat16, name="w2t")
            wl = wlp_pool.tile([128, 2 * PCOL], dt.bfloat16, name="wl")
            nc.sync.dma_start(
                out=wl[:, 0:PCOL], in_=wlp_d[:, 2 * lt * PCOL : (2 * lt + 1) * PCOL]
            )
            nc.sync.dma_start(
                out=wl[:, PCOL : 2 * PCOL],
                in_=wlp_d[:, (2 * lt + 1) * PCOL : (2 * lt + 2) * PCOL],
            )
            j = lt % 2
            w2_mms(j, wl, 0, odd=False, first=True)
            w2_copies(lt, j, (0, CIJ))
            final_block(lt - 1)
            # Even-half final runs while the odd half's DMA lands. The odd
            # matmuls then re-initialize the bank with start=True (their
            # rows 0-48 zeros overwrite the even data -- already copied out).
            final_half(lt, (0, CIJ), start=False, stop=False, pop=False)
            w2_mms(j, wl, PCOL, odd=True, first=True)
            w2_copies(lt, j, (64, ROWS))
            final_half(lt, (64, ROWS), start=False, stop=True, pop=True)

            for bh in range(2):
                ob = out_pool.tile([128, E], dt.bfloat16, tag=f"ob{bh}")
                nc.vector.tensor_copy(ob[:, 0:512], py[bh][0][:, 0:512])
                nc.scalar.copy(ob[:, 512:768], py[bh][1][:, 0:256])
                nc.scalar.dma_start(
                    out=out_d[bh * 128 : (bh + 1) * 128, :], in_=ob[:]
                )
    _split_extra_waits(nc)
    return nc


def _split_extra_waits(nc):
    """Walrus encodes at most one semaphore wait on regular engine
    instructions (Matmult, DMACopy, ...). When Tile attaches more (e.g.
    slot-recycle release + data-ready on different procs), split the extras
    onto InstEventSemaphore instructions inserted immediately before the
    instruction on the same engine queue -- semantically identical to the
    multi-wait (the engine blocks at the same point for all of them)."""
    import bass_rust
    import concourse.mybir as mybir

    keep_multi = {"InstEventSemaphore", "InstUnconditionalBranch"}
    n_split = 0
    for fn in nc.m.functions:
        for bb in fn.blocks:
            out = []
            changed = False
            for ins in bb.instructions:
                si = ins.sync_info
                if (
                    si is not None
                    and len(si.on_wait) > 1
                    and type(ins).__name__ not in keep_multi
                ):
                    waits = list(si.on_wait)
                    for w in waits[:-1]:
                        ev = mybir.InstEventSemaphore(
                            name=f"W-split-{n_split}", ins=[], outs=[]
                        )
                        n_split += 1
                        ev.engine = ins.engine
                        ev.sync_info = bass_rust.SyncInfo(on_wait=[w], on_update=[])
                        out.append(ev)
                    ins.sync_info = bass_rust.SyncInfo(
                        on_wait=[waits[-1]], on_update=list(si.on_update)
                    )
                    changed = True
                out.append(ins)
            if changed:
                bb.instructions = out
    return n_split


def _prep_inputs(x, wconv, bconv, wlin):
    bf16 = ml_dtypes.bfloat16
    x = np.ascontiguousarray(np.asarray(x, dtype=np.float32))
    wconv = np.asarray(wconv, dtype=np.float32)
    bconv = np.asarray(bconv, dtype=np.float32)
    wlin = np.asarray(wlin, dtype=np.float32)

    # im2col: xpa[(c,i,j), b, p] = x[b, c, 4hp+i, 4wp+j], p = hp*16+wp;
    # row 48 = ones (bias row). Pure index remap, zero FLOPs.
    xp = x.reshape(B, C, Hp, P, Wp, P).transpose(1, 3, 5, 0, 2, 4)
    xpa = np.empty((CIJ, B, NP), np.float32)
    xpa[:48] = xp.reshape(48, B, NP)
    xpa[48] = 1.0

    # wca[cij, e]: conv weights with bconv as row 48.
    wca = np.empty((CIJ, E), np.float32)
    wca[:48] = wconv.reshape(E, 48).T
    wca[48] = bconv
    wcaT = wca.T  # [E, CIJ]
    # wce[e_r, ech*49+cij] = wcaT[ech*128+e_r, cij]
    wce = np.ascontiguousarray(
        wcaT.reshape(NECH, 128, CIJ).transpose(1, 0, 2).reshape(128, NECH * CIJ)
    ).astype(bf16)
    # wco: zero-padded odd-patch stationary [128, 6*113]; columns 64-112
    # hold wcaT, columns 0-63 are zero so the matmul output lands at PSUM
    # rows 64-112 and adds +0 to rows 0-63.
    wcoF = np.zeros((NECH, 128, ROWS), np.float32)
    wcoF[:, :, 64 : 64 + CIJ] = wcaT.reshape(NECH, 128, CIJ)
    wco = np.ascontiguousarray(
        wcoF.transpose(1, 0, 2).reshape(128, NECH * ROWS)
    ).astype(bf16)

    wlinR = wlin.reshape(E, E, NP)  # [eo, e, p]
    in_maps = []
    for c in range(NCORES):
        ps = c * PL
        # wlp[e_r, p*4608 + ech*768 + eo] = wlin[eo, (ech*128+e_r)*256 + p]
        wlp = (
            wlinR[:, :, ps : ps + PL]
            .transpose(1, 2, 0)                 # [e, p, eo]
            .reshape(NECH, 128, PL, E)
            .transpose(1, 2, 0, 3)              # [e_r, p, ech, eo]
            .reshape(128, PL * PCOL)
            .astype(bf16)
        )
        # xpp[row, pair*256 + b]: rows 0-48 even patch, 49-63 zero,
        # 64-112 odd patch.
        xc = xpa[:, :, ps : ps + PL]            # [49, 256, 32]
        xpp = np.zeros((ROWS, NPAIR, B), np.float32)
        xpp[0:CIJ] = xc[:, :, 0::2].transpose(0, 2, 1)
        xpp[64 : 64 + CIJ] = xc[:, :, 1::2].transpose(0, 2, 1)
        xpp = xpp.reshape(ROWS, NPAIR * B).astype(bf16)
        in_maps.append({"wce": wce, "wco": wco, "xpp": xpp, "wlp": wlp})
    return in_maps


def _run(x, wconv, bconv, wlin, blin, trace=False, **trace_kwargs):
    from concourse.bass_utils import run_bass_kernel_spmd

    if "nc" not in _CACHE:
        _CACHE["nc"] = _build_bass()
    in_maps = _prep_inputs(x, wconv, bconv, wlin)
    res = run_bass_kernel_spmd(
        _CACHE["nc"], in_maps, core_ids=list(range(NCORES)), trace=trace,
        **trace_kwargs,
    )
    acc = np.zeros((B, E), np.float64)
    for r in res.results:
        acc += np.asarray(r["y"], dtype=np.float64)
    y = (acc + np.asarray(blin, dtype=np.float64)[None, :]).astype(np.float32)
    return y, res


def kernel(x, wconv, bconv, wlin, blin, patch_size):
    assert int(patch_size) == P
    y, _ = _run(x, wconv, bconv, wlin, blin, trace=False)
    return y


# revision 30
# speedup vs baseline: 1.1413x; 1.1413x over previous
"""Trainium2 Bass kernel: patch-conv (Conv2d C3->E768, k4 s4) + giant linear.

y[b, eo] = sum_K flat[b, K] * wlin[eo, K] + blin[eo],
flat[b, e*256+p] = conv[b, e, p] (+ bconv[e]), K = 196608.

Reassociated algorithm (matrix-chain reordering, all FLOPs on device):
    y[b,eo] = sum_{p,cij} xp[cij,p,b] * W2[p,cij,eo]
    W2[p,cij,eo] = sum_e wca[cij,e] * wlin[eo, e*256+p]
where xp is the im2col of x (pure index remap, row 48 = ones) and wca is
wconv reshaped [49, E] (row 48 = bconv). This computes the identical
function with 19.7 GFLOP instead of 82 GFLOP, and reads wlin exactly once.

Sharding (8 cores): shard the patch index p (32 patches/core). Each core:
  - streams its wlin slice (host re-layout wlp[e_r, (p, ech, eo)] bf16,
    37.75 MB -- the HBM roofline term) on the sync HWDGE queue,
  - W2-mm per patch pair: the even patch uses stationary wce [128,49]
    (PSUM rows 0-48); the odd patch uses wce_odd [128,113] whose first 64
    columns are zero, so its result lands at PSUM rows 64-112 of the SAME
    bank while rows 0-63 accumulate +0 (never-written-region trick),
  - copies the pair's W2 [113, 768] to SBUF bf16 (rows 49-63 are zeros),
  - final-mm contracts over all 113 rows at once: xpp2 packs the even
    patch's im2col at rows 0-48, zeros at 49-63, odd at 64-112 -> one
    matmul accumulates BOTH patches into y-partial PSUM (halves the
    final-mm column count vs per-patch matmuls).
  - The last two patches run as singles to minimize the end-of-stream
    serial chain. Output partial-y is cast bf16; host sums cores + blin.
All PSUM tiles are bank-sized (2048B): 4 y banks + 2x2 W2 banks = 8.
"""

import numpy as np
import ml_dtypes

B, C, H, W = 256, 3, 64, 64
P, Hp, Wp, NP = 4, 16, 16, 256
E = 768
CIJ = 49                  # 48 conv taps + 1 bias row
ROWS = 113                # packed pair rows: 49 even + 15 zero + 49 odd
NCORES = 8
PL = NP // NCORES         # 32 patches per core
NPAIR = PL // 2           # 16 patch pairs
NECH = E // 128           # 6 e-chunks in the W2 contraction
PCOL = NECH * E           # 4608 wlp columns per patch

_CACHE = {}


def _build_bass():
    import concourse.bass as bass
    import concourse.mybir as mybir
    import concourse.tile as tile
    from contextlib import ExitStack

    dt = mybir.dt
    nc = bass.Bass()
    wce_d = nc.dram_tensor("wce", [128, NECH * CIJ], dt.bfloat16, kind="ExternalInput")
    wco_d = nc.dram_tensor("wco", [128, NECH * ROWS], dt.bfloat16, kind="ExternalInput")
    xpp_d = nc.dram_tensor("xpp", [ROWS, NPAIR * B], dt.bfloat16, kind="ExternalInput")
    wlp_d = nc.dram_tensor("wlp", [128, PL * PCOL], dt.bfloat16, kind="ExternalInput")
    out_d = nc.dram_tensor("y", [B, E], dt.bfloat16, kind="ExternalOutput")

    with tile.TileContext(nc) as tc, ExitStack() as ctx:
        singles = ctx.enter_context(tc.tile_pool(name="singles", bufs=1))
        # The wlp stream owns the sync HWDGE queue; small loads and outputs
        # ride gpsimd so the copy engines (vector/scalar) issue no DMAs.
        wce = singles.tile([128, NECH * CIJ], dt.bfloat16)
        nc.gpsimd.dma_start(out=wce[:], in_=wce_d[:])
        wco = singles.tile([128, NECH * ROWS], dt.bfloat16)
        nc.gpsimd.dma_start(out=wco[:], in_=wco_d[:])
        xpp = singles.tile([ROWS, NPAIR * B], dt.bfloat16)
        half = NPAIR * B // 2
        nc.gpsimd.dma_start(out=xpp[:, 0:half], in_=xpp_d[:, 0:half])
        nc.gpsimd.dma_start(out=xpp[:, half:], in_=xpp_d[:, half:])

        # W2 staging ring: six explicit [113, 768] bf16 tiles. Rows 49-63
        # (between the even and odd patches' data) are memset to zero ONCE;
        # per-pair copies never touch them, so the packed final matmul
        # always contracts well-defined zeros there.
        w2ring = [
            singles.tile([ROWS, E], dt.bfloat16, tag=f"w2s{i}", name=f"w2s{i}")
            for i in range(6)
        ]
        for t in w2ring:
            nc.gpsimd.memset(t[32:64, :], 0.0)

        wlp_pool = ctx.enter_context(tc.tile_pool(name="wlp", bufs=4))
        out_pool = ctx.enter_context(tc.tile_pool(name="out_sb", bufs=1))

        # Warmup: absorb the wce/wco DMA-ready waits on throwaway matmuls so
        # the first real W2 matmul carries only the wlp(0) wait.
        with tc.tile_pool(name="psum_w", bufs=1, space="PSUM") as pwarm:
            wm = pwarm.tile([CIJ, CIJ], dt.float32)
            nc.tensor.matmul(
                wm[:], wce[:, 0:CIJ], wce[:, 0:CIJ], start=True, stop=True,
                skip_group_check=True,
            )
            wm2 = pwarm.tile([ROWS, CIJ], dt.float32, tag="wm2", name="wm2")
            nc.tensor.matmul(
                wm2[:], wco[:, 0:ROWS], wce[:, 0:CIJ], start=True, stop=True,
                skip_group_check=True,
            )

        with (
            tc.tile_pool(name="psum_y", bufs=1, space="PSUM") as pyp,
            tc.tile_pool(name="psum_w2", bufs=1, space="PSUM") as ppw,
        ):
            # y-partial accumulators: [128b x 512eo] + [128b x 256eo] per
            # b-half; each tile is a full PSUM bank.
            py = [
                [
                    pyp.tile([128, 512], dt.float32, tag=f"py{bh}0", name=f"py{bh}0"),
                    pyp.tile([128, 512], dt.float32, tag=f"py{bh}1", name=f"py{bh}1"),
                ]
                for bh in range(2)
            ]

            w2tiles = {}
            # Four bank-sized W2 accumulators (two pairs in flight).
            pw = [
                [
                    ppw.tile([ROWS, 512], dt.float32, tag=f"pa{j}", name=f"pa{j}"),
                    ppw.tile([ROWS, 512], dt.float32, tag=f"pb{j}", name=f"pb{j}"),
                ]
                for j in range(2)
            ]

            def w2_mms(j, wl, base, odd, first):
                # One patch's 12 accumulating matmuls. odd=True places the
                # result at PSUM rows 64-112 via the zero-padded stationary;
                # its rows 0-63 add +0 to the even patch's region.
                for ech in range(NECH):
                    if odd:
                        lhsT = wco[:, ech * ROWS : (ech + 1) * ROWS]
                        pa, pb = pw[j][0][:, 0:512], pw[j][1][:, 0:256]
                    else:
                        lhsT = wce[:, ech * CIJ : (ech + 1) * CIJ]
                        pa, pb = pw[j][0][0:CIJ, 0:512], pw[j][1][0:CIJ, 0:256]
                    nc.tensor.matmul(
                        pa, lhsT, wl[:, base + ech * E : base + ech * E + 512],
                        start=(first and ech == 0),
                        stop=(ech == NECH - 1),
                        skip_group_check=True,
                    )
                    nc.tensor.matmul(
                        pb, lhsT, wl[:, base + ech * E + 512 : base + ech * E + 768],
                        start=(first and ech == 0),
                        stop=(ech == NECH - 1),
                        skip_group_check=True,
                    )

            def w2_copies(pt, j, rows, flip=False):
                # PSUM f32 -> SBUF bf16 cast-copies for row range `rows`,
                # one 512-col + one 256-col copy split across both engines.
                w2t = w2tiles[pt]
                lo, hi = rows
                e0, e1 = (nc.scalar.copy, nc.vector.tensor_copy)
                if flip:
                    e0, e1 = e1, e0
                e0(w2t[lo:hi, 0:512], pw[j][0][lo:hi, 0:512])
                e1(w2t[lo:hi, 512:768], pw[j][1][lo:hi, 0:256])

            def w2_pair_block(pt, wl):
                w2tiles[pt] = w2ring[pt % 6]
                j = pt % 2
                # The odd matmul goes FIRST with start=True: PSUM pending-zero
                # marking covers only the writing instruction's partitions, so
                # the [113, *] odd output initializes the full row range
                # (rows 0-63 get its zero-stationary zeros); the even matmuls
                # then accumulate onto rows 0-48.
                w2_mms(j, wl, PCOL, odd=True, first=True)
                w2_mms(j, wl, 0, odd=False, first=False)
                # Two partition windows (base 0 and base 64, both legal);
                # rows 49-63 stay at their memset zeros.
                w2_copies(pt, j, (0, CIJ))
                w2_copies(pt, j, (64, ROWS), flip=True)

            def final_half(pt, rows, start, stop, pop):
                # Finals for a row range of pair pt's packed W2 tile.
                w2t = w2tiles.pop(pt) if pop else w2tiles[pt]
                lo, hi = rows
                for bh in range(2):
                    lhsT = xpp[lo:hi, pt * B + bh * 128 : pt * B + bh * 128 + 128]
                    nc.tensor.matmul(
                        py[bh][0][:, 0:512],
                        lhsT,
                        w2t[lo:hi, 0:512],
                        start=start, stop=stop, skip_group_check=True,
                    )
                    nc.tensor.matmul(
                        py[bh][1][:, 0:256],
                        lhsT,
                        w2t[lo:hi, 512:768],
                        start=start, stop=stop, skip_group_check=True,
                    )

            def final_block(pt):
                # One matmul pair per b-half contracts BOTH patches of the
                # pair (rows 0-48 even, 64-112 odd, 49-63 zeros).
                final_half(pt, (0, ROWS), start=(pt == 0), stop=False, pop=True)

            # wlp streams as one sequential 2-patch DMA chain on the sync
            # HWDGE queue (18432B per-partition lines, one packet each); the
            # final blocks run one pair behind W2 so the PE never stalls on
            # the PSUM->SBUF copy round-trip. The last pair is split into
            # 1-patch DMAs/matmul chains to shorten the end-of-stream chain.
            for pt in range(NPAIR - 1):
                wl = wlp_pool.tile([128, 2 * PCOL], dt.bfloat16, name="wl")
                nc.sync.dma_start(
                    out=wl[:], in_=wlp_d[:, pt * 2 * PCOL : (pt + 1) * 2 * PCOL]
                )
                w2_pair_block(pt, wl)
                if pt >= 1:
                    final_block(pt - 1)

            # Tail: last pair as two 1-patch DMAs and split finals, so the
            # serial chain after the last wlp packet is just the odd half's
            # 12 matmuls + copy + 4 finals.
            lt = NPAIR - 1
            w2tiles[lt] = w2ring[lt % 6]
            wl = wlp_pool.tile([128, 2 * PCOL], dt.bfloat16, name="wl")
            nc.sync.dma_start(
                out=wl[:, 0:PCOL], in_=wlp_d[:, 2 * lt * PCOL : (2 * lt + 1) * PCOL]
            )
            nc.sync.dma_start(
                out=wl[:, PCOL : 2 * PCOL],
                in_=wlp_d[:, (2 * lt + 1) * PCOL : (2 * lt + 2) * PCOL],
            )
            j = lt % 2
            w2_mms(j, wl, 0, odd=False, first=True)
            w2_copies(lt, j, (0, CIJ))
            final_block(lt - 1)
            # Even-half final runs while the odd half's DMA lands. The odd
            # matmuls then re-initialize the bank with start=True (their
            # rows 0-48 zeros overwrite the even data -- already copied out).
            final_half(lt, (0, CIJ), start=False, stop=False, pop=False)
            w2_mms(j, wl, PCOL, odd=True, first=True)
            w2_copies(lt, j, (64, ROWS), flip=True)
            final_half(lt, (64, ROWS), start=False, stop=True, pop=True)

            for bh in range(2):
                ob = out_pool.tile([128, E], dt.bfloat16, tag=f"ob{bh}")
                nc.vector.tensor_copy(ob[:, 0:512], py[bh][0][:, 0:512])
                nc.scalar.copy(ob[:, 512:768], py[bh][1][:, 0:256])
                nc.scalar.dma_start(
                    out=out_d[bh * 128 : (bh + 1) * 128, :], in_=ob[:]
                )
    _split_extra_waits(nc)
    return nc


def _split_extra_waits(nc):
    """Walrus encodes at most one semaphore wait on regular engine
    instructions (Matmult, DMACopy, ...). When Tile attaches more (e.g.
    slot-recycle release + data-ready on different procs), split the extras
    onto InstEventSemaphore instructions inserted immediately before the
    instruction on the same engine queue -- semantically identical to the
    multi-wait (the engine blocks at the same point for all of them)."""
    import bass_rust
    import concourse.mybir as mybir

    keep_multi = {"InstEventSemaphore", "InstUnconditionalBranch"}
    n_split = 0
    for fn in nc.m.functions:
        for bb in fn.blocks:
            out = []
            changed = False
            for ins in bb.instructions:
                si = ins.sync_info
                if (
                    si is not None
                    and len(si.on_wait) > 1
                    and type(ins).__name__ not in keep_multi
                ):
                    waits = list(si.on_wait)
                    for w in waits[:-1]:
                        ev = mybir.InstEventSemaphore(
                            name=f"W-split-{n_split}", ins=[], outs=[]
                        )
                        n_split += 1
                        ev.engine = ins.engine
                        ev.sync_info = bass_rust.SyncInfo(on_wait=[w], on_update=[])
                        out.append(ev)
                    ins.sync_info = bass_rust.SyncInfo(
                        on_wait=[waits[-1]], on_update=list(si.on_update)
                    )
                    changed = True
                out.append(ins)
            if changed:
                bb.instructions = out
    return n_split


def _prep_inputs(x, wconv, bconv, wlin):
    bf16 = ml_dtypes.bfloat16
    x = np.ascontiguousarray(np.asarray(x, dtype=np.float32))
    wconv = np.asarray(wconv, dtype=np.float32)
    bconv = np.asarray(bconv, dtype=np.float32)
    wlin = np.asarray(wlin, dtype=np.float32)

    # im2col: xpa[(c,i,j), b, p] = x[b, c, 4hp+i, 4wp+j], p = hp*16+wp;
    # row 48 = ones (bias row). Pure index remap, zero FLOPs.
    xp = x.reshape(B, C, Hp, P, Wp, P).transpose(1, 3, 5, 0, 2, 4)
    xpa = np.empty((CIJ, B, NP), np.float32)
    xpa[:48] = xp.reshape(48, B, NP)
    xpa[48] = 1.0

    # wca[cij, e]: conv weights with bconv as row 48.
    wca = np.empty((CIJ, E), np.float32)
    wca[:48] = wconv.reshape(E, 48).T
    wca[48] = bconv
    wcaT = wca.T  # [E, CIJ]
    # wce[e_r, ech*49+cij] = wcaT[ech*128+e_r, cij]
    wce = np.ascontiguousarray(
        wcaT.reshape(NECH, 128, CIJ).transpose(1, 0, 2).reshape(128, NECH * CIJ)
    ).astype(bf16)
    # wco: zero-padded odd-patch stationary [128, 6*113]; columns 64-112
    # hold wcaT, columns 0-63 are zero so the matmul output lands at PSUM
    # rows 64-112 and adds +0 to rows 0-63.
    wcoF = np.zeros((NECH, 128, ROWS), np.float32)
    wcoF[:, :, 64 : 64 + CIJ] = wcaT.reshape(NECH, 128, CIJ)
    wco = np.ascontiguousarray(
        wcoF.transpose(1, 0, 2).reshape(128, NECH * ROWS)
    ).astype(bf16)

    wlinR = wlin.reshape(E, E, NP)  # [eo, e, p]
    in_maps = []
    for c in range(NCORES):
        ps = c * PL
        # wlp[e_r, p*4608 + ech*768 + eo] = wlin[eo, (ech*128+e_r)*256 + p]
        wlp = (
            wlinR[:, :, ps : ps + PL]
            .transpose(1, 2, 0)                 # [e, p, eo]
            .reshape(NECH, 128, PL, E)
            .transpose(1, 2, 0, 3)              # [e_r, p, ech, eo]
            .reshape(128, PL * PCOL)
            .astype(bf16)
        )
        # xpp[row, pair*256 + b]: rows 0-48 even patch, 49-63 zero,
        # 64-112 odd patch.
        xc = xpa[:, :, ps : ps + PL]            # [49, 256, 32]
        xpp = np.zeros((ROWS, NPAIR, B), np.float32)
        xpp[0:CIJ] = xc[:, :, 0::2].transpose(0, 2, 1)
        xpp[64 : 64 + CIJ] = xc[:, :, 1::2].transpose(0, 2, 1)
        xpp = xpp.reshape(ROWS, NPAIR * B).astype(bf16)
        in_maps.append({"wce": wce, "wco": wco, "xpp": xpp, "wlp": wlp})
    return in_maps


def _run(x, wconv, bconv, wlin, blin, trace=False, **trace_kwargs):
    from concourse.bass_utils import run_bass_kernel_spmd

    if "nc" not in _CACHE:
        _CACHE["nc"] = _build_bass()
    in_maps = _prep_inputs(x, wconv, bconv, wlin)
    res = run_bass_kernel_spmd(
        _CACHE["nc"], in_maps, core_ids=list(range(NCORES)), trace=trace,
        **trace_kwargs,
    )
    acc = np.zeros((B, E), np.float64)
    for r in res.results:
        acc += np.asarray(r["y"], dtype=np.float64)
    y = (acc + np.asarray(blin, dtype=np.float64)[None, :]).astype(np.float32)
    return y, res


def kernel(x, wconv, bconv, wlin, blin, patch_size):
    assert int(patch_size) == P
    y, _ = _run(x, wconv, bconv, wlin, blin, trace=False)
    return y
